# revision 22
# baseline (speedup 1.0000x reference)
"""Trainium2 Bass kernel for CustomGRU (B=64, T=512, D=512, U=1024).

Sharding: data-parallel over batch across 8 NeuronCores (8 rows each),
weights replicated (a per-step inter-core exchange is ruled out by the
~5-12us collective latency floor x 512 steps). Per core:

  Phase 1 (projections): xzr[t,b,:] = X[b,t,:] @ [Wz|Wr|Wh] + b  -> DRAM
    - stationary = X^T tiles (host-pre-transposed), moving = Wcat (f32r,
      1 cyc/row at N=512).
  Phase 2 (recurrence), per step t, all in B-major [8, u] except the
  matmul stationaries:
    - gate pre-activations h_{t-1} @ U via 4-way column-tiled PE
      streaming: h^T chunks [128,8] (zero-padded to M=32 slots) are
      stationary in four 32-column groups of the PE array
      (tile_position=(0,32g)); the fp16 U-weight slices [128,512] stream
      through 4 XBUSes concurrently, 2 rounds of 4 chunks accumulating
      into partition blocks 32g..32g+8 of one PSUM tile. Round-1 matmuls
      use start=True (the has_written clear is region-scoped). An
      "eye-matmul" accumulates xzr_t (kept f32r for precision) onto
      group 0. A copy + ones-pattern matmul reduces the 4 partition
      blocks to the [8,512] gate pre-activation.
    - sigmoid/tanh on ScalarE evict the reduced PSUM -> SBUF.
    - r is PE-transposed to U-major to form (r*h)^T, the stationary of
      the candidate matmul; h_new is PE-transposed back to h^T.
    - combine h = hh + z*(h_prev - hh) on VectorE; the tanh/combine/
      transpose/copy tail is split into 512-halves so the next step's
      round-1 matmuls (needing only h chunks 0-3) start early.

Weight matmuls run in fp16 (1 cyc/row, col-tiling compatible; ~2e-4
end-to-end rel err, same order as float32r); reductions, xz preloads
and projections in f32r; everything else fp32.
"""
import sys

if "/opt/trn_rl_repo" not in sys.path:
    sys.path.insert(0, "/opt/trn_rl_repo")

import numpy as np
from contextlib import ExitStack

import concourse.bass as bass
import concourse.bacc as bacc
import concourse.tile as tile
from concourse import mybir
from concourse.bass_utils import run_bass_kernel_spmd

F32 = mybir.dt.float32
F32R = mybir.dt.float32r
F16 = mybir.dt.float16

N_CORES = 8
B = 64
BS = B // N_CORES  # 8 batch rows per core
D = 512
U = 1024
U3 = 3 * U        # 3072 (z|r|h)
KC = U // 128     # 8 contraction chunks of 128
DC = D // 128     # 4 input-dim chunks


def r32(ap):
    return ap.bitcast(F32R)


def build(nc, T, reps=1):
    BT = BS * T

    # ---- DRAM I/O (per-core) ----
    xT_d = nc.dram_tensor("xT", [D, BT], F32R, kind="ExternalInput")
    wcat_d = nc.dram_tensor("wcat", [D, U3], F32R, kind="ExternalInput")
    bb_d = nc.dram_tensor("bb", [128, U3], F32, kind="ExternalInput")
    uzr_d = nc.dram_tensor("uzr", [U, 2 * U], F16, kind="ExternalInput")
    uh_d = nc.dram_tensor("uh", [U, U], F16, kind="ExternalInput")
    eye8r_d = nc.dram_tensor("eye8r", [BS, BS], F32R, kind="ExternalInput")
    ones4_d = nc.dram_tensor("ones4", [128, BS], F32R, kind="ExternalInput")
    eye8f_d = nc.dram_tensor("eye8f", [BS, BS], F32, kind="ExternalInput")
    out_d = nc.dram_tensor("out", [T, BS, U], F32, kind="ExternalOutput")

    with tile.TileContext(nc) as tc, ExitStack() as ctx:
        dram = ctx.enter_context(tc.tile_pool(name="dram", bufs=1, space="DRAM"))
        xzr_d = dram.tile([T, BS, U3], F32R)

        const = ctx.enter_context(tc.tile_pool(name="const", bufs=1))
        eye8r = const.tile([BS, BS], F32R)
        nc.sync.dma_start(eye8r[:], eye8r_d[:])
        ones4 = const.tile([128, BS], F32R)
        nc.sync.dma_start(ones4[:], ones4_d[:])
        eye8f = const.tile([BS, BS], F32)
        nc.sync.dma_start(eye8f[:], eye8f_d[:])

        # ---------------- Phase 1: input projections ----------------
        with ExitStack() as p1:
            wpool = p1.enter_context(tc.tile_pool(name="wcat", bufs=1))
            wcat = wpool.tile([128, DC * U3], F32R)  # [p, dc, u]
            nc.sync.dma_start(
                wcat[:].rearrange("p (dc u) -> p dc u", dc=DC),
                wcat_d.rearrange("(dc p) u -> p dc u", p=128),
            )
            bb = wpool.tile([128, U3], F32)
            nc.sync.dma_start(bb[:], bb_d[:])

            xp = p1.enter_context(tc.tile_pool(name="xT", bufs=3))
            op = p1.enter_context(tc.tile_pool(name="p1out", bufs=3))
            pp = p1.enter_context(tc.tile_pool(name="p1ps", bufs=4, space="PSUM"))

            n_bt = BT // 128          # bt-chunks of 128 (4 per batch row)
            tpb = T // 128            # t-chunks per batch row
            for tb in range(n_bt):
                b_idx, t_blk = tb // tpb, tb % tpb
                xt = xp.tile([128, DC * 128], F32R, tag="xt")  # [p=d, dc, bt]
                nc.sync.dma_start(
                    xt[:].rearrange("p (dc n) -> p dc n", dc=DC),
                    xT_d[:, tb * 128:(tb + 1) * 128].rearrange(
                        "(dc p) n -> p dc n", p=128
                    ),
                )
                for ut in range(U3 // 512):
                    ps = pp.tile([128, 512], F32, tag="ps")
                    for dc in range(DC):
                        nc.tensor.matmul(
                            ps[:],
                            xt[:, dc * 128:(dc + 1) * 128],
                            wcat[:, dc * U3 + ut * 512: dc * U3 + ut * 512 + 512],
                            start=(dc == 0),
                            stop=(dc == DC - 1),
                        )
                    ob = op.tile([128, 512], F32R, tag="ob")
                    nc.vector.tensor_add(
                        ob[:], ps[:], bb[:, ut * 512:(ut + 1) * 512]
                    )
                    nc.sync.dma_start(
                        xzr_d[
                            t_blk * 128:(t_blk + 1) * 128,
                            b_idx,
                            ut * 512:(ut + 1) * 512,
                        ].squeeze(),
                        ob[:],
                    )

        # ---------------- Phase 2: recurrence ----------------
        upool = ctx.enter_context(tc.tile_pool(name="u", bufs=1))
        uzr = upool.tile([128, KC * 2 * U], F16)  # [p, k, 2U]
        nc.sync.dma_start(
            uzr[:].rearrange("p (k u) -> p k u", k=KC),
            uzr_d.rearrange("(k p) u -> p k u", p=128),
        )
        uh = upool.tile([128, KC * U], F16)
        nc.sync.dma_start(
            uh[:].rearrange("p (k u) -> p k u", k=KC),
            uh_d.rearrange("(k p) u -> p k u", p=128),
        )

        hpool = ctx.enter_context(tc.tile_pool(name="h", bufs=2))
        stage = ctx.enter_context(tc.tile_pool(name="stage", bufs=4))
        gates = ctx.enter_context(tc.tile_pool(name="gates", bufs=2))
        psg = ctx.enter_context(tc.tile_pool(name="psg", bufs=4, space="PSUM"))
        ps2 = ctx.enter_context(tc.tile_pool(name="ps2", bufs=2, space="PSUM"))
        pst = ctx.enter_context(tc.tile_pool(name="pst", bufs=1, space="PSUM"))
        red = ctx.enter_context(tc.tile_pool(name="red", bufs=3))
        # zero the col-tiled psum slots once so untouched partition rows
        # (multiplied by 0 in the ones-reduction) are never uninitialized
        for _i in range(4):
            _d = psg.tile([128, 512], F32, tag="psg")
            nc.vector.memset(_d[:], 0.0)

        # h^T chunks in 32-col padded slots (cols 32k..32k+8 hold chunk k,
        # rest zero) so col-tiled matmuls use M=32 stationaries.
        hT0 = const.tile([128, KC * 32], F16)
        nc.any.memzero(hT0[:])
        hT_prev = hT0
        # pre-zero the rT/hTps psum slots once: per-step transposes write
        # only the 8 valid cols of each 32-col slot; the full-width mul/copy
        # reads the (zero) pads
        _c = pst.tile([128, KC * 32], F32, tag="rT")
        nc.vector.memset(_c[:], 0.0)
        _e = pst.tile([128, KC * 32], F32, tag="hTps")
        nc.vector.memset(_e[:], 0.0)

        def gate_mms(xoff, uoff, umat, hT, tag, copy_eng):
            """Two [8,512] reduced psum tiles via 4-way col-tiled streaming.

            Each [8,512] gate tile: eye-MM preloads xz into partitions 0-8
            (start=True clears the bank), then 8 K-chunk matmuls run on 4
            col-groups (tile_position=(0,32g), 2 rounds) writing partials to
            partition blocks 32g..32g+8. A copy + ones-pattern matmul sums
            the 4 blocks (+xz) back to [8,512]."""
            tiles = []
            for j in range(2):
                ps = psg.tile([128, 512], F32, tag="psg")
                xz = xz_t[:, xoff + 512 * j: xoff + 512 * j + 512]
                # round 1 (chunks 0-3): start=True so each group clears its
                # own 32-row psum region (has_written clear is region-scoped)
                for k in range(KC):
                    g = k % 4
                    nc.tensor.matmul(
                        ps[32 * g:32 * g + 32, :],
                        hT[:, k * 32:(k + 1) * 32],
                        umat[:, k * WSTRIDE + uoff + 512 * j:
                             k * WSTRIDE + uoff + 512 * j + 512],
                        start=(k < 4),
                        stop=(k == KC - 1),
                        tile_position=(0, 32 * g),
                        skip_group_check=True,
                    )
                    if k == 3:
                        # xz preload accumulates onto group 0's rows 0-8
                        nc.tensor.matmul(ps[0:BS, :], eye8r[:], xz,
                                         start=False, stop=False,
                                         tile_position=(0, 0),
                                         skip_group_check=True)
                sb = red.tile([128, 512], F32R, tag="red")
                if copy_eng == "act":
                    nc.scalar.copy(sb[:], ps[:])
                else:
                    nc.vector.tensor_copy(sb[:], ps[:])
                pr = ps2.tile([BS, 512], F32, tag="ps2")
                nc.tensor.matmul(pr[:], ones4[:], sb[:], start=True, stop=True)
                tiles.append(pr)
            return tiles

        for rep in range(reps):
          for t in range(T):
            xz_t = stage.tile([BS, U3], F32R, tag="xz")
            nc.sync.dma_start(xz_t[:], xzr_d[t].squeeze())

            # r gate, then transpose to U-major and form (r*h)^T
            WSTRIDE = 2 * U
            ps_r = gate_mms(U, U, uzr, hT_prev, "r", "dve")
            # z gate (keeps PE busy while sigmoid(r) runs)
            ps_z = gate_mms(0, 0, uzr, hT_prev, "z", "act")
            r_B = gates.tile([BS, U], F32, tag="r")
            rT = pst.tile([128, KC * 32], F32, tag="rT")
            rhT = hpool.tile([128, KC * 32], F16, tag="rhT")
            for j in range(2):
                sl = slice(512 * j, 512 * j + 512)
                nc.scalar.activation(
                    r_B[:, sl], ps_r[j][:],
                    mybir.ActivationFunctionType.Sigmoid,
                )
                for c in range(4 * j, 4 * j + 4):
                    nc.tensor.transpose(
                        rT[:, c * 32:c * 32 + BS],
                        r_B[:, c * 128:(c + 1) * 128],
                        eye8f[:],
                    )
                nc.vector.tensor_mul(
                    rhT[:, 128 * j:128 * (j + 1)],
                    rT[:, 128 * j:128 * (j + 1)],
                    hT_prev[:, 128 * j:128 * (j + 1)])
            z_B = gates.tile([BS, U], F32, tag="z")
            for j in range(2):
                nc.scalar.activation(
                    z_B[:, 512 * j:512 * j + 512], ps_z[j][:],
                    mybir.ActivationFunctionType.Sigmoid,
                )

            # candidate
            WSTRIDE = U
            ps_h = gate_mms(2 * U, 0, uh, rhT, "hh", "act")
            if t == 0:
                h_B_prev = gates.tile([BS, U], F32, tag="hB")
                nc.any.memzero(h_B_prev[:])
            hh_B = gates.tile([BS, U], F32, tag="hh")
            h_B = gates.tile([BS, U], F32, tag="hB")
            hT_ps = pst.tile([128, KC * 32], F32, tag="hTps")
            hT_new = hpool.tile([128, KC * 32], F16, tag="hT")
            # per 512-half: tanh -> combine -> transpose -> h^T copy, so the
            # next step's round-1 matmuls (which read only h chunks 0-3)
            # start while this half-1 is still combining
            for j in range(2):
                sl = slice(512 * j, 512 * j + 512)
                nc.scalar.activation(
                    hh_B[:, sl], ps_h[j][:],
                    mybir.ActivationFunctionType.Tanh,
                )
                # combine: h = hh + z * (h_prev - hh)   (B-major, VectorE)
                tmp = gates.tile([BS, 512], F32, tag="tmp")
                nc.vector.tensor_sub(tmp[:], h_B_prev[:, sl], hh_B[:, sl])
                nc.vector.tensor_mul(tmp[:], z_B[:, sl], tmp[:])
                nc.vector.tensor_add(h_B[:, sl], hh_B[:, sl], tmp[:])
                for c in range(4 * j, 4 * j + 4):
                    nc.tensor.transpose(
                        hT_ps[:, c * 32:c * 32 + BS],
                        h_B[:, c * 128:(c + 1) * 128],
                        eye8f[:],
                    )
                nc.vector.tensor_copy(
                    hT_new[:, 128 * j:128 * (j + 1)],
                    hT_ps[:, 128 * j:128 * (j + 1)])

            nc.sync.dma_start(out_d[t].squeeze(), h_B[:])
            hT_prev = hT_new
            h_B_prev = h_B

    nc.compile()
    return nc


def prepare(inputs, Wz, Uz, bz, Wr, Ur, br, Wh, Uh, bh, T):
    """Build the Bass program and the per-core input maps."""
    x = np.asarray(inputs, dtype=np.float32)[:, :T, :]

    wcat = np.concatenate([Wz, Wr, Wh], axis=1).astype(np.float32)
    bcat = np.concatenate([bz, br, bh]).astype(np.float32)
    bb = np.ascontiguousarray(np.broadcast_to(bcat, (128, U3)))
    uzr = np.concatenate([Uz, Ur], axis=1).astype(np.float16)
    uh = np.asarray(Uh).astype(np.float16)
    eye8 = np.eye(BS, dtype=np.float32)
    ones4 = np.zeros((128, BS), dtype=np.float32)
    for g in range(4):
        for b in range(BS):
            ones4[32 * g + b, b] = 1.0

    nc = bacc.Bacc("TRN2", target_bir_lowering=False, debug=False,
                   num_devices=N_CORES)
    build(nc, T)

    in_maps = []
    for c in range(N_CORES):
        xc = x[c * BS:(c + 1) * BS]               # [BS, T, D]
        xT = np.ascontiguousarray(xc.reshape(BS * T, D).T)  # [D, BS*T]
        in_maps.append({
            "xT": xT, "wcat": wcat, "bb": bb, "uzr": uzr, "uh": uh,
            "eye8r": eye8, "eye8f": eye8, "ones4": ones4,
        })
    return nc, in_maps


def assemble(results):
    outs = []
    for c in range(N_CORES):
        o = results[c]["out"]                     # [T, BS, U]
        outs.append(np.ascontiguousarray(o.transpose(1, 0, 2)))
    return np.concatenate(outs, axis=0)           # [B, T, U]


def kernel(inputs, Wz, Uz, bz, Wr, Ur, br, Wh, Uh, bh, _T=None):
    T = inputs.shape[1] if _T is None else _T
    nc, in_maps = prepare(inputs, Wz, Uz, bz, Wr, Ur, br, Wh, Uh, bh, T)
    res = run_bass_kernel_spmd(nc, in_maps, list(range(N_CORES)))
    return assemble(res.results)


# revision 23
# speedup vs baseline: 10.0829x; 10.0829x over previous
"""Trainium2 Bass kernel for CustomGRU (B=64, T=512, D=512, U=1024).

Sharding: data-parallel over batch across 8 NeuronCores (8 rows each),
weights replicated (a per-step inter-core exchange is ruled out by the
~5-12us collective latency floor x 512 steps). Per core:

  Phase 1 (projections): xzr[t,b,:] = X[b,t,:] @ [Wz|Wr|Wh] + b  -> DRAM
    - stationary = X^T tiles (host-pre-transposed), moving = Wcat (f32r,
      1 cyc/row at N=512).
  Phase 2 (recurrence), per step t, all in B-major [8, u] except the
  matmul stationaries:
    - gate pre-activations h_{t-1} @ U via 4-way column-tiled PE
      streaming: h^T chunks [128,8] (zero-padded to M=32 slots) are
      stationary in four 32-column groups of the PE array
      (tile_position=(0,32g)); the fp16 U-weight slices [128,512] stream
      through 4 XBUSes concurrently, 2 rounds of 4 chunks accumulating
      into partition blocks 32g..32g+8 of one PSUM tile. Round-1 matmuls
      use start=True (the has_written clear is region-scoped). An
      "eye-matmul" accumulates xzr_t (kept f32r for precision) onto
      group 0. A copy + ones-pattern matmul reduces the 4 partition
      blocks to the [8,512] gate pre-activation.
    - sigmoid/tanh on ScalarE evict the reduced PSUM -> SBUF.
    - r is PE-transposed to U-major to form (r*h)^T, the stationary of
      the candidate matmul; h_new is PE-transposed back to h^T.
    - combine h = hh + z*(h_prev - hh) on VectorE; the tanh/combine/
      transpose/copy tail is split into 512-halves so the next step's
      round-1 matmuls (needing only h chunks 0-3) start early.

Weight matmuls run in fp16 (1 cyc/row, col-tiling compatible; ~2e-4
end-to-end rel err, same order as float32r); reductions, xz preloads
and projections in f32r; everything else fp32.
"""
import sys

if "/opt/trn_rl_repo" not in sys.path:
    sys.path.insert(0, "/opt/trn_rl_repo")

import numpy as np
from contextlib import ExitStack

import concourse.bass as bass
import concourse.bacc as bacc
import concourse.tile as tile
from concourse import mybir
from concourse.bass_utils import run_bass_kernel_spmd

F32 = mybir.dt.float32
F32R = mybir.dt.float32r
F16 = mybir.dt.float16

N_CORES = 8
B = 64
BS = B // N_CORES  # 8 batch rows per core
D = 512
U = 1024
U3 = 3 * U        # 3072 (z|r|h)
KC = U // 128     # 8 contraction chunks of 128
DC = D // 128     # 4 input-dim chunks


def build(nc, T, reps=1):
    BT = BS * T

    # ---- DRAM I/O (per-core) ----
    xT_d = nc.dram_tensor("xT", [D, BT], F32R, kind="ExternalInput")
    wcat_d = nc.dram_tensor("wcat", [D, U3], F32R, kind="ExternalInput")
    bb_d = nc.dram_tensor("bb", [128, U3], F32, kind="ExternalInput")
    uzr_d = nc.dram_tensor("uzr", [U, 2 * U], F16, kind="ExternalInput")
    uh_d = nc.dram_tensor("uh", [U, U], F16, kind="ExternalInput")
    eye8r_d = nc.dram_tensor("eye8r", [BS, BS], F32R, kind="ExternalInput")
    ones4_d = nc.dram_tensor("ones4", [128, BS], F32R, kind="ExternalInput")
    eye8f_d = nc.dram_tensor("eye8f", [BS, BS], F32, kind="ExternalInput")
    out_d = nc.dram_tensor("out", [T, BS, U], F32, kind="ExternalOutput")

    with tile.TileContext(nc) as tc, ExitStack() as ctx:
        dram = ctx.enter_context(tc.tile_pool(name="dram", bufs=1, space="DRAM"))
        xzr_d = dram.tile([T, BS, U3], F32R)

        const = ctx.enter_context(tc.tile_pool(name="const", bufs=1))
        eye8r = const.tile([BS, BS], F32R)
        nc.sync.dma_start(eye8r[:], eye8r_d[:])
        ones4 = const.tile([128, BS], F32R)
        nc.sync.dma_start(ones4[:], ones4_d[:])
        eye8f = const.tile([BS, BS], F32)
        nc.sync.dma_start(eye8f[:], eye8f_d[:])

        # ---------------- Phase 1: input projections ----------------
        with ExitStack() as p1:
            wpool = p1.enter_context(tc.tile_pool(name="wcat", bufs=1))
            wcat = wpool.tile([128, DC * U3], F32R)  # [p, dc, u]
            nc.sync.dma_start(
                wcat[:].rearrange("p (dc u) -> p dc u", dc=DC),
                wcat_d.rearrange("(dc p) u -> p dc u", p=128),
            )
            bb = wpool.tile([128, U3], F32)
            nc.sync.dma_start(bb[:], bb_d[:])

            xp = p1.enter_context(tc.tile_pool(name="xT", bufs=3))
            op = p1.enter_context(tc.tile_pool(name="p1out", bufs=3))
            pp = p1.enter_context(tc.tile_pool(name="p1ps", bufs=4, space="PSUM"))

            n_bt = BT // 128          # bt-chunks of 128 (4 per batch row)
            tpb = T // 128            # t-chunks per batch row
            for tb in range(n_bt):
                b_idx, t_blk = tb // tpb, tb % tpb
                xt = xp.tile([128, DC * 128], F32R, tag="xt")  # [p=d, dc, bt]
                nc.sync.dma_start(
                    xt[:].rearrange("p (dc n) -> p dc n", dc=DC),
                    xT_d[:, tb * 128:(tb + 1) * 128].rearrange(
                        "(dc p) n -> p dc n", p=128
                    ),
                )
                for ut in range(U3 // 512):
                    ps = pp.tile([128, 512], F32, tag="ps")
                    for dc in range(DC):
                        nc.tensor.matmul(
                            ps[:],
                            xt[:, dc * 128:(dc + 1) * 128],
                            wcat[:, dc * U3 + ut * 512: dc * U3 + ut * 512 + 512],
                            start=(dc == 0),
                            stop=(dc == DC - 1),
                        )
                    ob = op.tile([128, 512], F32R, tag="ob")
                    nc.vector.tensor_add(
                        ob[:], ps[:], bb[:, ut * 512:(ut + 1) * 512]
                    )
                    nc.sync.dma_start(
                        xzr_d[
                            t_blk * 128:(t_blk + 1) * 128,
                            b_idx,
                            ut * 512:(ut + 1) * 512,
                        ].squeeze(),
                        ob[:],
                    )

        # ---------------- Phase 2: recurrence ----------------
        upool = ctx.enter_context(tc.tile_pool(name="u", bufs=1))
        uzr = upool.tile([128, KC * 2 * U], F16)  # [p, k, 2U]
        nc.sync.dma_start(
            uzr[:].rearrange("p (k u) -> p k u", k=KC),
            uzr_d.rearrange("(k p) u -> p k u", p=128),
        )
        uh = upool.tile([128, KC * U], F16)
        nc.sync.dma_start(
            uh[:].rearrange("p (k u) -> p k u", k=KC),
            uh_d.rearrange("(k p) u -> p k u", p=128),
        )

        hpool = ctx.enter_context(tc.tile_pool(name="h", bufs=2))
        stage = ctx.enter_context(tc.tile_pool(name="stage", bufs=4))
        gates = ctx.enter_context(tc.tile_pool(name="gates", bufs=2))
        psg = ctx.enter_context(tc.tile_pool(name="psg", bufs=4, space="PSUM"))
        ps2 = ctx.enter_context(tc.tile_pool(name="ps2", bufs=2, space="PSUM"))
        pst = ctx.enter_context(tc.tile_pool(name="pst", bufs=1, space="PSUM"))
        red = ctx.enter_context(tc.tile_pool(name="red", bufs=3))
        # zero the col-tiled psum slots once so untouched partition rows
        # (multiplied by 0 in the ones-reduction) are never uninitialized
        for _i in range(4):
            _d = psg.tile([128, 512], F32, tag="psg")
            nc.vector.memset(_d[:], 0.0)

        # h^T chunks in 32-col padded slots (cols 32k..32k+8 hold chunk k,
        # rest zero) so col-tiled matmuls use M=32 stationaries.
        hT0 = const.tile([128, KC * 32], F16)
        nc.any.memzero(hT0[:])
        hT_prev = hT0
        # pre-zero the rT/hTps psum slots once: per-step transposes write
        # only the 8 valid cols of each 32-col slot; the full-width mul/copy
        # reads the (zero) pads
        _c = pst.tile([128, KC * 32], F32, tag="rT")
        nc.vector.memset(_c[:], 0.0)
        _e = pst.tile([128, KC * 32], F32, tag="hTps")
        nc.vector.memset(_e[:], 0.0)

        def gate_mms(xoff, uoff, umat, hT, tag, copy_eng):
            """Two [8,512] reduced psum tiles via 4-way col-tiled streaming.

            Each [8,512] gate tile: eye-MM preloads xz into partitions 0-8
            (start=True clears the bank), then 8 K-chunk matmuls run on 4
            col-groups (tile_position=(0,32g), 2 rounds) writing partials to
            partition blocks 32g..32g+8. A copy + ones-pattern matmul sums
            the 4 blocks (+xz) back to [8,512]."""
            tiles = []
            for j in range(2):
                ps = psg.tile([128, 512], F32, tag="psg")
                xz = xz_t[:, xoff + 512 * j: xoff + 512 * j + 512]
                # round 1 (chunks 0-3): start=True so each group clears its
                # own 32-row psum region (has_written clear is region-scoped)
                for k in range(KC):
                    g = k % 4
                    nc.tensor.matmul(
                        ps[32 * g:32 * g + 32, :],
                        hT[:, k * 32:(k + 1) * 32],
                        umat[:, k * WSTRIDE + uoff + 512 * j:
                             k * WSTRIDE + uoff + 512 * j + 512],
                        start=(k < 4),
                        stop=(k == KC - 1),
                        tile_position=(0, 32 * g),
                        skip_group_check=True,
                    )
                    if k == 3:
                        # xz preload accumulates onto group 0's rows 0-8
                        nc.tensor.matmul(ps[0:BS, :], eye8r[:], xz,
                                         start=False, stop=False,
                                         tile_position=(0, 0),
                                         skip_group_check=True)
                sb = red.tile([128, 512], F32R, tag="red")
                if copy_eng == "act":
                    nc.scalar.copy(sb[:], ps[:])
                else:
                    nc.vector.tensor_copy(sb[:], ps[:])
                pr = ps2.tile([BS, 512], F32, tag="ps2")
                nc.tensor.matmul(pr[:], ones4[:], sb[:], start=True, stop=True)
                tiles.append(pr)
            return tiles

        for rep in range(reps):
          for t in range(T):
            xz_t = stage.tile([BS, U3], F32R, tag="xz")
            nc.sync.dma_start(xz_t[:], xzr_d[t].squeeze())

            # r gate, then transpose to U-major and form (r*h)^T
            WSTRIDE = 2 * U
            ps_r = gate_mms(U, U, uzr, hT_prev, "r", "dve")
            # z gate (keeps PE busy while sigmoid(r) runs)
            ps_z = gate_mms(0, 0, uzr, hT_prev, "z", "act")
            r_B = gates.tile([BS, U], F32, tag="r")
            rT = pst.tile([128, KC * 32], F32, tag="rT")
            rhT = hpool.tile([128, KC * 32], F16, tag="rhT")
            for j in range(2):
                sl = slice(512 * j, 512 * j + 512)
                nc.scalar.activation(
                    r_B[:, sl], ps_r[j][:],
                    mybir.ActivationFunctionType.Sigmoid,
                )
                for c in range(4 * j, 4 * j + 4):
                    nc.tensor.transpose(
                        rT[:, c * 32:c * 32 + BS],
                        r_B[:, c * 128:(c + 1) * 128],
                        eye8f[:],
                    )
                nc.vector.tensor_mul(
                    rhT[:, 128 * j:128 * (j + 1)],
                    rT[:, 128 * j:128 * (j + 1)],
                    hT_prev[:, 128 * j:128 * (j + 1)])
            z_B = gates.tile([BS, U], F32, tag="z")
            for j in range(2):
                nc.scalar.activation(
                    z_B[:, 512 * j:512 * j + 512], ps_z[j][:],
                    mybir.ActivationFunctionType.Sigmoid,
                )

            # candidate
            WSTRIDE = U
            ps_h = gate_mms(2 * U, 0, uh, rhT, "hh", "act")
            if t == 0:
                h_B_prev = gates.tile([BS, U], F32, tag="hB")
                nc.any.memzero(h_B_prev[:])
            hh_B = gates.tile([BS, U], F32, tag="hh")
            h_B = gates.tile([BS, U], F32, tag="hB")
            hT_ps = pst.tile([128, KC * 32], F32, tag="hTps")
            hT_new = hpool.tile([128, KC * 32], F16, tag="hT")
            # per 512-half: tanh -> combine -> transpose -> h^T copy, so the
            # next step's round-1 matmuls (which read only h chunks 0-3)
            # start while this half-1 is still combining
            for j in range(2):
                sl = slice(512 * j, 512 * j + 512)
                nc.scalar.activation(
                    hh_B[:, sl], ps_h[j][:],
                    mybir.ActivationFunctionType.Tanh,
                )
                # combine: h = hh + z * (h_prev - hh)   (B-major, VectorE)
                tmp = gates.tile([BS, 512], F32, tag="tmp")
                nc.vector.tensor_sub(tmp[:], h_B_prev[:, sl], hh_B[:, sl])
                nc.vector.tensor_mul(tmp[:], z_B[:, sl], tmp[:])
                nc.vector.tensor_add(h_B[:, sl], hh_B[:, sl], tmp[:])
                for c in range(4 * j, 4 * j + 4):
                    nc.tensor.transpose(
                        hT_ps[:, c * 32:c * 32 + BS],
                        h_B[:, c * 128:(c + 1) * 128],
                        eye8f[:],
                    )
                nc.vector.tensor_copy(
                    hT_new[:, 128 * j:128 * (j + 1)],
                    hT_ps[:, 128 * j:128 * (j + 1)])

            nc.sync.dma_start(out_d[t].squeeze(), h_B[:])
            hT_prev = hT_new
            h_B_prev = h_B

    nc.compile()
    return nc


def prepare(inputs, Wz, Uz, bz, Wr, Ur, br, Wh, Uh, bh, T):
    """Build the Bass program and the per-core input maps."""
    x = np.asarray(inputs, dtype=np.float32)[:, :T, :]

    wcat = np.concatenate([Wz, Wr, Wh], axis=1).astype(np.float32)
    bcat = np.concatenate([bz, br, bh]).astype(np.float32)
    bb = np.ascontiguousarray(np.broadcast_to(bcat, (128, U3)))
    uzr = np.concatenate([Uz, Ur], axis=1).astype(np.float16)
    uh = np.asarray(Uh).astype(np.float16)
    eye8 = np.eye(BS, dtype=np.float32)
    ones4 = np.zeros((128, BS), dtype=np.float32)
    for g in range(4):
        for b in range(BS):
            ones4[32 * g + b, b] = 1.0

    nc = bacc.Bacc("TRN2", target_bir_lowering=False, debug=False,
                   num_devices=N_CORES)
    build(nc, T)

    in_maps = []
    for c in range(N_CORES):
        xc = x[c * BS:(c + 1) * BS]               # [BS, T, D]
        xT = np.ascontiguousarray(xc.reshape(BS * T, D).T)  # [D, BS*T]
        in_maps.append({
            "xT": xT, "wcat": wcat, "bb": bb, "uzr": uzr, "uh": uh,
            "eye8r": eye8, "eye8f": eye8, "ones4": ones4,
        })
    return nc, in_maps


def assemble(results):
    outs = []
    for c in range(N_CORES):
        o = results[c]["out"]                     # [T, BS, U]
        outs.append(np.ascontiguousarray(o.transpose(1, 0, 2)))
    return np.concatenate(outs, axis=0)           # [B, T, U]


def kernel(inputs, Wz, Uz, bz, Wr, Ur, br, Wh, Uh, bh, _T=None):
    T = inputs.shape[1] if _T is None else _T
    nc, in_maps = prepare(inputs, Wz, Uz, bz, Wr, Ur, br, Wh, Uh, bh, T)
    res = run_bass_kernel_spmd(nc, in_maps, list(range(N_CORES)))
    return assemble(res.results)
